# revision 23
# baseline (speedup 1.0000x reference)
"""AttentionAggregator kernel for 8 trn2 NeuronCores.

Math (linearity of the shared feat_weights matmul + wa-prescaling):
  wa = feat_weights @ attn_weights                       # [128,1]
  host passes ns = neigh*wa, ss = self*wa (bf16), W' = W/wa (bf16)
  logit[n,k]  = sum_d ns[n,k,d] ; sl[n] = sum_d ss[n,d]
  E[n,k]      = exp(leaky_relu(logit+sl, 0.2))
  agg'[n,:]   = sum_k E[n,k] * ns[n,k,:]
  out[n,:]    = relu((ss[n,:] + agg'[n,:]/sum_k E[n,k]) @ W' + bias)

Per-tile device pipeline (128 nodes):
  - row-sums via step-0 accumulating out-AP matmuls (lhsT = identity)
  - softmax pieces on DVE/ACT, diag(E_k) built on DVE+ACT+POOL
  - combine = 32 diag matmuls accumulating in PSUM
  - final: transpose (Sn) on PE, matmul with W', bias seed matmul, relu on ACT
"""

import sys

sys.path.insert(0, "/opt/trn_rl_repo")

import numpy as np
import ml_dtypes

import concourse.bass as bass
import concourse.bacc as bacc
import concourse.mybir as mybir
import concourse.tile as tile
from concourse.bass_utils import run_bass_kernel_spmd

N_CORES = 8
D = 128
K = 32
P = 128
TILES = 49
NODES_PC = TILES * P             # 6272
ROWS_PC = NODES_PC * K           # 200704
N_FULL = 50000

F32 = mybir.dt.float32
BF16 = mybir.dt.bfloat16
BF = ml_dtypes.bfloat16

_cache = {}


def _build():
    nc = bacc.Bacc("TRN2", target_bir_lowering=False, debug=False)

    self_t = nc.dram_tensor("self_sh", [NODES_PC, D], BF16, kind="ExternalInput")
    neigh_t = nc.dram_tensor("neigh_sh", [ROWS_PC, D], BF16, kind="ExternalInput")
    w_t = nc.dram_tensor("w_bf", [D, D], BF16, kind="ExternalInput")
    ident_t = nc.dram_tensor("ident_bf", [P, P], BF16, kind="ExternalInput")
    ones_t = nc.dram_tensor("ones_bf", [1, P], BF16, kind="ExternalInput")
    bias_t = nc.dram_tensor("bias_bf", [1, D], BF16, kind="ExternalInput")
    out_t = nc.dram_tensor("out", [NODES_PC, D], F32, kind="ExternalOutput")

    with tile.TileContext(nc) as tc:
        with (
            tc.tile_pool(name="const", bufs=1) as cpool,
            tc.tile_pool(name="big", bufs=1) as bigpool,
            tc.tile_pool(name="nb", bufs=4) as nbpool,
            tc.tile_pool(name="work", bufs=3) as wpool,
            tc.tile_pool(name="small", bufs=6) as smpool,
            tc.tile_pool(name="ps_log", bufs=2, space="PSUM") as ps_log,
            tc.tile_pool(name="ps_agg", bufs=2, space="PSUM") as ps_agg,
            tc.tile_pool(name="ps_fin", bufs=1, space="PSUM") as ps_fin,
        ):
            ident = cpool.tile([P, P], BF16)
            allones = cpool.tile([P, P], BF16)
            w_sb = cpool.tile([D, D], BF16)
            ones_sb = cpool.tile([1, P], BF16)
            bias_sb = cpool.tile([1, D], BF16)
            nc.sync.dma_start(ident[:], ident_t[:])
            nc.gpsimd.memset(allones[:], 1.0)
            nc.sync.dma_start(w_sb[:], w_t[:])
            nc.sync.dma_start(ones_sb[:], ones_t[:])
            nc.sync.dma_start(bias_sb[:], bias_t[:])

            self_sb = bigpool.tile([P, TILES * D], BF16)
            for t in range(TILES):
                nc.sync.dma_start(
                    self_sb[:, t * D : (t + 1) * D], self_t[t * P : (t + 1) * P, :]
                )

            for t in range(TILES):
                nb = nbpool.tile([P, K * D], BF16, tag="nb")
                nc.sync.dma_start(
                    nb[:],
                    neigh_t[t * P * K : (t + 1) * P * K, :].rearrange(
                        "(p c) d -> p (c d)", p=P
                    ),
                )
                sf = self_sb[:, t * D : (t + 1) * D]

                # ---- logits via bf16 pair-split step-0 matmuls ----
                log_ps = ps_log.tile([P, 2 * K], F32, tag="log_ps")
                for g in range(8):
                    out_ap = (
                        log_ps[:, g * 8 : (g + 1) * 8]
                        .rearrange("p (kk r) -> p kk r", r=2)
                        .unsqueeze(2)
                        .broadcast_to((P, 4, D // 2, 2))
                    )
                    nc.tensor.matmul(
                        out_ap, ident[:], nb[:, g * 4 * D : (g + 1) * 4 * D]
                    )

                # ---- self logit via DVE accumulate ----
                junk = smpool.tile([P, D], BF16, tag="junk")
                sl = smpool.tile([P, 1], F32, tag="sl")
                nc.vector.scalar_tensor_tensor(
                    junk[:], sf, 1.0, allones[:],
                    mybir.AluOpType.mult, mybir.AluOpType.mult,
                    accum_out=sl[:],
                )

                # ---- E = exp(leaky(logit + sl)); pair-sum via reduce ----
                p_sb = smpool.tile([P, K], F32, tag="p_sb")
                nc.vector.tensor_reduce(
                    p_sb[:],
                    log_ps[:].rearrange("p (kk r) -> p kk r", r=2),
                    axis=mybir.AxisListType.X,
                    op=mybir.AluOpType.add,
                )
                a_sb = smpool.tile([P, K], F32, tag="a_sb")
                nc.vector.tensor_scalar_add(a_sb[:], p_sb[:], sl[:])
                l_sb = smpool.tile([P, K], F32, tag="l_sb")
                nc.vector.scalar_tensor_tensor(
                    l_sb[:], a_sb[:], 0.2, a_sb[:],
                    mybir.AluOpType.mult, mybir.AluOpType.max,
                )
                e_sb = smpool.tile([P, K], F32, tag="e_sb")
                nc.scalar.activation(e_sb[:], l_sb[:], mybir.ActivationFunctionType.Exp)

                s_sb = smpool.tile([P, 1], F32, tag="s_sb")
                nc.vector.tensor_reduce(
                    s_sb[:], e_sb[:], axis=mybir.AxisListType.X, op=mybir.AluOpType.add
                )
                r_sb = smpool.tile([P, 1], F32, tag="r_sb")
                nc.vector.reciprocal(r_sb[:], s_sb[:])

                # ---- scale nb by E on three engines; step-0 combine on PE ----
                sc = wpool.tile([P, K * D], BF16, tag="sc")
                for k in range(K):
                    dk = sc[:, k * D : (k + 1) * D]
                    nk = nb[:, k * D : (k + 1) * D]
                    ek = e_sb[:, k : k + 1]
                    m = k % 4
                    if m < 2:
                        nc.vector.tensor_scalar_mul(dk, nk, ek)
                    elif m < 3:
                        nc.scalar.activation(
                            dk, nk, mybir.ActivationFunctionType.Copy, scale=ek
                        )
                    else:
                        nc.gpsimd.tensor_scalar_mul(dk, nk, ek)
                agg_ps = ps_agg.tile([P, D], F32, tag="agg_ps")
                for k in range(K):
                    nc.tensor.matmul(
                        agg_ps[:],
                        ident[:],
                        sc[:, k * D : (k + 1) * D],
                        start=(k == 0),
                        stop=(k == K - 1),
                    )

                # ---- Sn = ss + R*agg' ; transpose; @W' + bias; relu ----
                sn_sb = smpool.tile([P, D], BF16, tag="sn_sb")
                nc.vector.scalar_tensor_tensor(
                    sn_sb[:], agg_ps[:], r_sb[:], sf,
                    mybir.AluOpType.mult, mybir.AluOpType.add,
                )
                snt_ps = ps_fin.tile([P, D], F32, tag="snt_ps")
                nc.tensor.matmul(snt_ps[:], sn_sb[:], ident[:])
                snt_sb = smpool.tile([P, D], BF16, tag="snt_sb")
                nc.scalar.copy(snt_sb[:], snt_ps[:])

                o_ps = ps_fin.tile([P, D], F32, tag="o_ps")
                nc.tensor.matmul(o_ps[:], ones_sb[:], bias_sb[:], start=True, stop=False)
                nc.tensor.matmul(o_ps[:], snt_sb[:], w_sb[:], start=False, stop=True)
                o_sb = smpool.tile([P, D], F32, tag="o_sb")
                nc.scalar.activation(
                    o_sb[:], o_ps[:], mybir.ActivationFunctionType.Relu
                )
                nc.sync.dma_start(out_t[t * P : (t + 1) * P, :], o_sb[:])

    nc.compile()
    return nc


def _prep(self_vecs, neigh_vecs, feat_weights, attn_weights, bias):
    n = self_vecs.shape[0]
    n_pad = N_CORES * NODES_PC
    wa = (feat_weights.astype(np.float64) @ attn_weights.astype(np.float64)).reshape(
        1, D
    )
    self_p = np.zeros((n_pad, D), BF)
    self_p[:n] = (self_vecs.astype(np.float64) * wa).astype(BF)
    neigh_p = np.zeros((n_pad * K, D), BF)
    neigh_p[: n * K] = (neigh_vecs.astype(np.float64) * wa).astype(BF)
    w_p = (feat_weights.astype(np.float64) / wa.reshape(D, 1)).astype(BF)
    return self_p, neigh_p, w_p


def kernel(self_vecs, neigh_vecs, feat_weights, attn_weights, bias, num_neighbors):
    self_vecs = np.asarray(self_vecs, dtype=np.float32)
    neigh_vecs = np.asarray(neigh_vecs, dtype=np.float32)
    feat_weights = np.asarray(feat_weights, dtype=np.float32)
    attn_weights = np.asarray(attn_weights, dtype=np.float32)
    bias = np.asarray(bias, dtype=np.float32)
    n = self_vecs.shape[0]

    self_p, neigh_p, w_p = _prep(
        self_vecs, neigh_vecs, feat_weights, attn_weights, bias
    )
    mk = {
        "w_bf": w_p,
        "ident_bf": np.eye(P, dtype=np.float32).astype(BF),
        "ones_bf": np.ones((1, P), np.float32).astype(BF),
        "bias_bf": bias.reshape(1, D).astype(BF),
    }

    if "nc" not in _cache:
        _cache["nc"] = _build()
    nc = _cache["nc"]

    in_maps = []
    for c in range(N_CORES):
        m = {
            "self_sh": self_p[c * NODES_PC : (c + 1) * NODES_PC],
            "neigh_sh": neigh_p[c * ROWS_PC : (c + 1) * ROWS_PC],
        }
        m.update(mk)
        in_maps.append(m)

    import os

    trace = os.environ.get("KERNEL_TRACE") == "1"
    res = run_bass_kernel_spmd(nc, in_maps, list(range(N_CORES)), trace=trace)
    _cache["last_result"] = res
    out = np.concatenate([res.results[c]["out"] for c in range(N_CORES)], axis=0)
    return out[:n].astype(np.float32)


# revision 30
# speedup vs baseline: 16.5503x; 16.5503x over previous
"""AttentionAggregator kernel for 8 trn2 NeuronCores.

Math (linearity of the shared feat_weights matmul + wa-prescaling):
  wa = feat_weights @ attn_weights                       # [128,1]
  host passes ns = neigh*wa, ss = self*wa (bf16), W' = W/wa (bf16)
  logit[n,k]  = sum_d ns[n,k,d] ; sl[n] = sum_d ss[n,d]
  E[n,k]      = exp(leaky_relu(logit+sl, 0.2))
  agg'[n,:]   = sum_k E[n,k] * ns[n,k,:]
  out[n,:]    = relu((ss[n,:] + agg'[n,:]/sum_k E[n,k]) @ W' + bias)

Per-tile device pipeline (128 nodes):
  - row-sums via step-0 accumulating out-AP matmuls (lhsT = identity)
  - softmax pieces on DVE/ACT, diag(E_k) built on DVE+ACT+POOL
  - combine = 32 diag matmuls accumulating in PSUM
  - final: transpose (Sn) on PE, matmul with W', bias seed matmul, relu on ACT
"""

import sys

sys.path.insert(0, "/opt/trn_rl_repo")

import numpy as np
import ml_dtypes

import concourse.bass as bass
import concourse.bacc as bacc
import concourse.mybir as mybir
import concourse.tile as tile
from concourse.bass_utils import run_bass_kernel_spmd

N_CORES = 8
D = 128
K = 32
P = 128
TILES = 49
NODES_PC = TILES * P             # 6272
ROWS_PC = NODES_PC * K           # 200704
N_FULL = 50000

F32 = mybir.dt.float32
BF16 = mybir.dt.bfloat16
BF = ml_dtypes.bfloat16

_cache = {}


def _build(reps=1, skip=()):
    nc = bacc.Bacc("TRN2", target_bir_lowering=False, debug=False)

    self_t = nc.dram_tensor("self_sh", [NODES_PC, D], BF16, kind="ExternalInput")
    neigh_t = nc.dram_tensor("neigh_sh", [ROWS_PC, D], BF16, kind="ExternalInput")
    w_t = nc.dram_tensor("w_bf", [D, D], BF16, kind="ExternalInput")
    ident_t = nc.dram_tensor("ident_bf", [P, P], BF16, kind="ExternalInput")
    ones_t = nc.dram_tensor("ones_bf", [1, P], BF16, kind="ExternalInput")
    bias_t = nc.dram_tensor("bias_bf", [1, D], BF16, kind="ExternalInput")
    out_t = nc.dram_tensor("out", [NODES_PC, D], F32, kind="ExternalOutput")

    with tile.TileContext(nc) as tc:
        with (
            tc.tile_pool(name="const", bufs=1) as cpool,
            tc.tile_pool(name="big", bufs=1) as bigpool,
            tc.tile_pool(name="nb", bufs=4) as nbpool,
            tc.tile_pool(name="work", bufs=3) as wpool,
            tc.tile_pool(name="small", bufs=6) as smpool,
            tc.tile_pool(name="ps_log", bufs=2, space="PSUM") as ps_log,
            tc.tile_pool(name="ps_agg", bufs=2, space="PSUM") as ps_agg,
            tc.tile_pool(name="ps_fin", bufs=1, space="PSUM") as ps_fin,
        ):
            ident = cpool.tile([P, P], BF16)
            allones = cpool.tile([P, P], BF16)
            w_sb = cpool.tile([D, D], BF16)
            ones_sb = cpool.tile([1, P], BF16)
            bias_sb = cpool.tile([1, D], BF16)
            nc.sync.dma_start(ident[:], ident_t[:])
            nc.gpsimd.memset(allones[:], 1.0)
            nc.sync.dma_start(w_sb[:], w_t[:])
            nc.sync.dma_start(ones_sb[:], ones_t[:])
            nc.sync.dma_start(bias_sb[:], bias_t[:])

            self_sb = bigpool.tile([P, TILES * D], BF16)
            out_big = bigpool.tile([P, TILES * D], F32)
            t0 = 0
            while t0 < TILES:
                q = min(4, TILES - t0)
                nc.sync.dma_start(
                    self_sb[:, t0 * D : (t0 + q) * D].rearrange(
                        "p (q d) -> p q d", q=q
                    ),
                    self_t[t0 * P : (t0 + q) * P, :].rearrange(
                        "(q p) d -> p q d", p=P
                    ),
                )
                t0 += q

            for rep in range(reps):
              for t in range(TILES):
                nb = nbpool.tile([P, K * D], BF16, tag="nb")
                if "dma" in skip and t > 0 and rep > 0:
                    pass
                else:
                    nc.sync.dma_start(
                        nb[:],
                        neigh_t[t * P * K : (t + 1) * P * K, :].rearrange(
                            "(p c) d -> p (c d)", p=P
                        ),
                    )
                sf = self_sb[:, t * D : (t + 1) * D]

                # ---- logits via bf16 pair-split step-0 matmuls ----
                log_ps = ps_log.tile([P, 2 * K], F32, tag="log_ps")
                for g in range(8 if "logits" not in skip else 1):
                    out_ap = (
                        log_ps[:, g * 8 : (g + 1) * 8]
                        .rearrange("p (kk r) -> p kk r", r=2)
                        .unsqueeze(2)
                        .broadcast_to((P, 4, D // 2, 2))
                    )
                    nc.tensor.matmul(
                        out_ap, ident[:], nb[:, g * 4 * D : (g + 1) * 4 * D]
                    )

                # ---- self logit via DVE accumulate ----
                junk = smpool.tile([P, D], BF16, tag="junk")
                sl = smpool.tile([P, 1], F32, tag="sl")
                nc.vector.scalar_tensor_tensor(
                    junk[:], sf, 1.0, allones[:],
                    mybir.AluOpType.mult, mybir.AluOpType.mult,
                    accum_out=sl[:],
                )

                # ---- E = exp(leaky(logit + sl)); pair-sum via reduce ----
                p_sb = smpool.tile([P, K], F32, tag="p_sb")
                nc.vector.tensor_reduce(
                    p_sb[:],
                    log_ps[:].rearrange("p (kk r) -> p kk r", r=2),
                    axis=mybir.AxisListType.X,
                    op=mybir.AluOpType.add,
                )
                a_sb = smpool.tile([P, K], F32, tag="a_sb")
                nc.vector.tensor_scalar_add(a_sb[:], p_sb[:], sl[:])
                l_sb = smpool.tile([P, K], F32, tag="l_sb")
                nc.vector.scalar_tensor_tensor(
                    l_sb[:], a_sb[:], 0.2, a_sb[:],
                    mybir.AluOpType.mult, mybir.AluOpType.max,
                )
                e_sb = smpool.tile([P, K], F32, tag="e_sb")
                nc.scalar.activation(e_sb[:], l_sb[:], mybir.ActivationFunctionType.Exp)

                s_sb = smpool.tile([P, 1], F32, tag="s_sb")
                nc.vector.tensor_reduce(
                    s_sb[:], e_sb[:], axis=mybir.AxisListType.X, op=mybir.AluOpType.add
                )
                r_sb = smpool.tile([P, 1], F32, tag="r_sb")
                nc.vector.reciprocal(r_sb[:], s_sb[:])

                # ---- scale nb by E on three engines; step-0 combine on PE ----
                sc = wpool.tile([P, K * D], BF16, tag="sc")
                for g in range(4 if "scale" not in skip else 1):
                    ebc = (
                        e_sb[:, g * 8 : (g + 1) * 8]
                        .unsqueeze(2)
                        .broadcast_to((P, 8, D))
                    )
                    nc.vector.tensor_tensor(
                        sc[:, g * 8 * D : (g + 1) * 8 * D],
                        nb[:, g * 8 * D : (g + 1) * 8 * D],
                        ebc,
                        mybir.AluOpType.mult,
                    )
                agg_ps = ps_agg.tile([P, D], F32, tag="agg_ps")
                for k in range(K if "combine" not in skip else 1):
                    nc.tensor.matmul(
                        agg_ps[:],
                        ident[:],
                        sc[:, k * D : (k + 1) * D],
                        start=(k == 0),
                        stop=True,
                    )

                # ---- Sn = ss + R*agg' ; transpose; @W' + bias; relu ----
                sn_sb = smpool.tile([P, D], BF16, tag="sn_sb")
                nc.vector.scalar_tensor_tensor(
                    sn_sb[:], agg_ps[:], r_sb[:], sf,
                    mybir.AluOpType.mult, mybir.AluOpType.add,
                )
                snt_ps = ps_fin.tile([P, D], F32, tag="snt_ps")
                nc.tensor.matmul(snt_ps[:], sn_sb[:], ident[:])
                snt_sb = smpool.tile([P, D], BF16, tag="snt_sb")
                nc.scalar.copy(snt_sb[:], snt_ps[:])

                o_ps = ps_fin.tile([P, D], F32, tag="o_ps")
                nc.tensor.matmul(o_ps[:], ones_sb[:], bias_sb[:], start=True, stop=False)
                nc.tensor.matmul(o_ps[:], snt_sb[:], w_sb[:], start=False, stop=True)
                nc.scalar.activation(
                    out_big[:, t * D : (t + 1) * D], o_ps[:],
                    mybir.ActivationFunctionType.Relu,
                )
                if t % 4 == 3 or t == TILES - 1:
                    t0g = (t // 4) * 4
                    qg = t - t0g + 1
                    nc.sync.dma_start(
                        out_t[t0g * P : (t0g + qg) * P, :].rearrange(
                            "(q p) d -> p q d", p=P
                        ),
                        out_big[:, t0g * D : (t + 1) * D].rearrange(
                            "p (q d) -> p q d", q=qg
                        ),
                    )

    nc.compile()
    return nc


def _prep(self_vecs, neigh_vecs, feat_weights, attn_weights, bias):
    n = self_vecs.shape[0]
    n_pad = N_CORES * NODES_PC
    wa = (feat_weights.astype(np.float64) @ attn_weights.astype(np.float64)).reshape(
        1, D
    )
    self_p = np.zeros((n_pad, D), BF)
    self_p[:n] = (self_vecs.astype(np.float64) * wa).astype(BF)
    neigh_p = np.zeros((n_pad * K, D), BF)
    neigh_p[: n * K] = (neigh_vecs.astype(np.float64) * wa).astype(BF)
    w_p = (feat_weights.astype(np.float64) / wa.reshape(D, 1)).astype(BF)
    return self_p, neigh_p, w_p


def kernel(self_vecs, neigh_vecs, feat_weights, attn_weights, bias, num_neighbors):
    self_vecs = np.asarray(self_vecs, dtype=np.float32)
    neigh_vecs = np.asarray(neigh_vecs, dtype=np.float32)
    feat_weights = np.asarray(feat_weights, dtype=np.float32)
    attn_weights = np.asarray(attn_weights, dtype=np.float32)
    bias = np.asarray(bias, dtype=np.float32)
    n = self_vecs.shape[0]

    self_p, neigh_p, w_p = _prep(
        self_vecs, neigh_vecs, feat_weights, attn_weights, bias
    )
    mk = {
        "w_bf": w_p,
        "ident_bf": np.eye(P, dtype=np.float32).astype(BF),
        "ones_bf": np.ones((1, P), np.float32).astype(BF),
        "bias_bf": bias.reshape(1, D).astype(BF),
    }

    if "nc" not in _cache:
        _cache["nc"] = _build()
    nc = _cache["nc"]

    in_maps = []
    for c in range(N_CORES):
        m = {
            "self_sh": self_p[c * NODES_PC : (c + 1) * NODES_PC],
            "neigh_sh": neigh_p[c * ROWS_PC : (c + 1) * ROWS_PC],
        }
        m.update(mk)
        in_maps.append(m)

    import os

    trace = os.environ.get("KERNEL_TRACE") == "1"
    res = run_bass_kernel_spmd(nc, in_maps, list(range(N_CORES)), trace=trace)
    _cache["last_result"] = res
    out = np.concatenate([res.results[c]["out"] for c in range(N_CORES)], axis=0)
    return out[:n].astype(np.float32)


# revision 32
# speedup vs baseline: 32.3432x; 1.9542x over previous
"""AttentionAggregator kernel for 8 trn2 NeuronCores.

Math (linearity of the shared feat_weights matmul + wa-prescaling):
  wa = feat_weights @ attn_weights                       # [128,1]
  host passes ns = neigh*wa, ss = self*wa (bf16), W' = W/wa (bf16)
  logit[n,k]  = sum_d ns[n,k,d] ; sl[n] = sum_d ss[n,d]
  E[n,k]      = exp(leaky_relu(logit+sl, 0.2))
  agg'[n,:]   = sum_k E[n,k] * ns[n,k,:]
  out[n,:]    = relu((ss[n,:] + agg'[n,:]/sum_k E[n,k]) @ W' + bias)

Per-tile device pipeline (128 nodes):
  - row-sums via step-0 accumulating out-AP matmuls (lhsT = identity)
  - softmax pieces on DVE/ACT, diag(E_k) built on DVE+ACT+POOL
  - combine = 32 diag matmuls accumulating in PSUM
  - final: transpose (Sn) on PE, matmul with W', bias seed matmul, relu on ACT
"""

import sys

sys.path.insert(0, "/opt/trn_rl_repo")

import numpy as np
import ml_dtypes

import concourse.bass as bass
import concourse.bacc as bacc
import concourse.mybir as mybir
import concourse.tile as tile
from concourse.bass_utils import run_bass_kernel_spmd

N_CORES = 8
D = 128
K = 32
P = 128
TILES = 49
NODES_PC = TILES * P             # 6272
ROWS_PC = NODES_PC * K           # 200704
N_FULL = 50000

F32 = mybir.dt.float32
BF16 = mybir.dt.bfloat16
BF = ml_dtypes.bfloat16

_cache = {}


def _build(reps=1, skip=()):
    nc = bacc.Bacc("TRN2", target_bir_lowering=False, debug=False)

    self_t = nc.dram_tensor("self_sh", [NODES_PC, D], BF16, kind="ExternalInput")
    neigh_t = nc.dram_tensor("neigh_sh", [ROWS_PC, D], BF16, kind="ExternalInput")
    w_t = nc.dram_tensor("w_bf", [D, D], BF16, kind="ExternalInput")
    ident_t = nc.dram_tensor("ident_bf", [P, P], BF16, kind="ExternalInput")
    ones_t = nc.dram_tensor("ones_bf", [1, P], BF16, kind="ExternalInput")
    bias_t = nc.dram_tensor("bias_bf", [1, D], BF16, kind="ExternalInput")
    out_t = nc.dram_tensor("out", [NODES_PC, D], F32, kind="ExternalOutput")

    with tile.TileContext(nc) as tc:
        with (
            tc.tile_pool(name="const", bufs=1) as cpool,
            tc.tile_pool(name="big", bufs=1) as bigpool,
            tc.tile_pool(name="nb", bufs=6) as nbpool,
            tc.tile_pool(name="work", bufs=4) as wpool,
            tc.tile_pool(name="small", bufs=6) as smpool,
            tc.tile_pool(name="ps_log", bufs=2, space="PSUM") as ps_log,
            tc.tile_pool(name="ps_agg", bufs=2, space="PSUM") as ps_agg,
            tc.tile_pool(name="ps_fin", bufs=1, space="PSUM") as ps_fin,
        ):
            ident = cpool.tile([P, P], BF16)
            allones = cpool.tile([P, P], BF16)
            w_sb = cpool.tile([D, D], BF16)
            ones_sb = cpool.tile([1, P], BF16)
            bias_sb = cpool.tile([1, D], BF16)
            nc.sync.dma_start(ident[:], ident_t[:])
            nc.gpsimd.memset(allones[:], 1.0)
            nc.sync.dma_start(w_sb[:], w_t[:])
            nc.sync.dma_start(ones_sb[:], ones_t[:])
            nc.sync.dma_start(bias_sb[:], bias_t[:])

            self_sb = bigpool.tile([P, TILES * D], BF16)
            out_big = bigpool.tile([P, TILES * D], F32)
            t0 = 0
            while t0 < TILES:
                q = min(4, TILES - t0)
                nc.sync.dma_start(
                    self_sb[:, t0 * D : (t0 + q) * D].rearrange(
                        "p (q d) -> p q d", q=q
                    ),
                    self_t[t0 * P : (t0 + q) * P, :].rearrange(
                        "(q p) d -> p q d", p=P
                    ),
                )
                t0 += q

            for rep in range(reps):
              for t in range(TILES):
                nb = nbpool.tile([P, K * D], BF16, tag="nb")
                if "dma" in skip and t > 0 and rep > 0:
                    pass
                else:
                    nc.sync.dma_start(
                        nb[:],
                        neigh_t[t * P * K : (t + 1) * P * K, :].rearrange(
                            "(p c) d -> p (c d)", p=P
                        ),
                    )
                sf = self_sb[:, t * D : (t + 1) * D]

                # ---- logits via bf16 pair-split step-0 matmuls ----
                log_ps = ps_log.tile([P, 2 * K], F32, tag="log_ps")
                for g in range(8 if "logits" not in skip else 1):
                    out_ap = (
                        log_ps[:, g * 8 : (g + 1) * 8]
                        .rearrange("p (kk r) -> p kk r", r=2)
                        .unsqueeze(2)
                        .broadcast_to((P, 4, D // 2, 2))
                    )
                    nc.tensor.matmul(
                        out_ap, ident[:], nb[:, g * 4 * D : (g + 1) * 4 * D]
                    )

                # ---- self logit via DVE accumulate ----
                junk = smpool.tile([P, D], BF16, tag="junk")
                sl = smpool.tile([P, 1], F32, tag="sl")
                nc.vector.scalar_tensor_tensor(
                    junk[:], sf, 1.0, allones[:],
                    mybir.AluOpType.mult, mybir.AluOpType.mult,
                    accum_out=sl[:],
                )

                # ---- E = exp(leaky(logit + sl)); pair-sum via reduce ----
                p_sb = smpool.tile([P, K], F32, tag="p_sb")
                nc.vector.tensor_reduce(
                    p_sb[:],
                    log_ps[:].rearrange("p (kk r) -> p kk r", r=2),
                    axis=mybir.AxisListType.X,
                    op=mybir.AluOpType.add,
                )
                a_sb = smpool.tile([P, K], F32, tag="a_sb")
                nc.vector.tensor_scalar_add(a_sb[:], p_sb[:], sl[:])
                l_sb = smpool.tile([P, K], F32, tag="l_sb")
                nc.vector.scalar_tensor_tensor(
                    l_sb[:], a_sb[:], 0.2, a_sb[:],
                    mybir.AluOpType.mult, mybir.AluOpType.max,
                )
                e_sb = smpool.tile([P, K], F32, tag="e_sb")
                nc.scalar.activation(e_sb[:], l_sb[:], mybir.ActivationFunctionType.Exp)

                s_sb = smpool.tile([P, 1], F32, tag="s_sb")
                nc.vector.tensor_reduce(
                    s_sb[:], e_sb[:], axis=mybir.AxisListType.X, op=mybir.AluOpType.add
                )
                r_sb = smpool.tile([P, 1], F32, tag="r_sb")
                nc.vector.reciprocal(r_sb[:], s_sb[:])

                # ---- scale nb by E on three engines; step-0 combine on PE ----
                sc = wpool.tile([P, K * D], BF16, tag="sc")
                for g in range(4 if "scale" not in skip else 1):
                    ebc = (
                        e_sb[:, g * 8 : (g + 1) * 8]
                        .unsqueeze(2)
                        .broadcast_to((P, 8, D))
                    )
                    nc.vector.tensor_tensor(
                        sc[:, g * 8 * D : (g + 1) * 8 * D],
                        nb[:, g * 8 * D : (g + 1) * 8 * D],
                        ebc,
                        mybir.AluOpType.mult,
                    )
                agg_ps = ps_agg.tile([P, D], F32, tag="agg_ps")
                for k in range(K if "combine" not in skip else 1):
                    nc.tensor.matmul(
                        agg_ps[:],
                        ident[:],
                        sc[:, k * D : (k + 1) * D],
                        start=(k == 0),
                        stop=True,
                    )

                # ---- Sn = ss + R*agg' ; transpose; @W' + bias; relu ----
                sn_sb = smpool.tile([P, D], BF16, tag="sn_sb")
                nc.vector.scalar_tensor_tensor(
                    sn_sb[:], agg_ps[:], r_sb[:], sf,
                    mybir.AluOpType.mult, mybir.AluOpType.add,
                )
                snt_ps = ps_fin.tile([P, D], F32, tag="snt_ps")
                nc.tensor.matmul(snt_ps[:], sn_sb[:], ident[:])
                snt_sb = smpool.tile([P, D], BF16, tag="snt_sb")
                nc.scalar.copy(snt_sb[:], snt_ps[:])

                o_ps = ps_fin.tile([P, D], F32, tag="o_ps")
                nc.tensor.matmul(o_ps[:], ones_sb[:], bias_sb[:], start=True, stop=False)
                nc.tensor.matmul(o_ps[:], snt_sb[:], w_sb[:], start=False, stop=True)
                nc.scalar.activation(
                    out_big[:, t * D : (t + 1) * D], o_ps[:],
                    mybir.ActivationFunctionType.Relu,
                )
                if t % 4 == 3 or t == TILES - 1:
                    t0g = (t // 4) * 4
                    qg = t - t0g + 1
                    nc.sync.dma_start(
                        out_t[t0g * P : (t0g + qg) * P, :].rearrange(
                            "(q p) d -> p q d", p=P
                        ),
                        out_big[:, t0g * D : (t + 1) * D].rearrange(
                            "p (q d) -> p q d", q=qg
                        ),
                    )

    nc.compile()
    return nc


def _prep(self_vecs, neigh_vecs, feat_weights, attn_weights, bias):
    n = self_vecs.shape[0]
    n_pad = N_CORES * NODES_PC
    wa = (feat_weights.astype(np.float64) @ attn_weights.astype(np.float64)).reshape(
        1, D
    )
    wa32 = wa.astype(np.float32)
    self_p = np.zeros((n_pad, D), BF)
    self_p[:n] = (self_vecs * wa32).astype(BF)
    neigh_p = np.zeros((n_pad * K, D), BF)
    neigh_p[: n * K] = (neigh_vecs * wa32).astype(BF)
    w_p = (feat_weights.astype(np.float64) / wa.reshape(D, 1)).astype(BF)
    return self_p, neigh_p, w_p


def kernel(self_vecs, neigh_vecs, feat_weights, attn_weights, bias, num_neighbors):
    self_vecs = np.asarray(self_vecs, dtype=np.float32)
    neigh_vecs = np.asarray(neigh_vecs, dtype=np.float32)
    feat_weights = np.asarray(feat_weights, dtype=np.float32)
    attn_weights = np.asarray(attn_weights, dtype=np.float32)
    bias = np.asarray(bias, dtype=np.float32)
    n = self_vecs.shape[0]

    self_p, neigh_p, w_p = _prep(
        self_vecs, neigh_vecs, feat_weights, attn_weights, bias
    )
    mk = {
        "w_bf": w_p,
        "ident_bf": np.eye(P, dtype=np.float32).astype(BF),
        "ones_bf": np.ones((1, P), np.float32).astype(BF),
        "bias_bf": bias.reshape(1, D).astype(BF),
    }

    if "nc" not in _cache:
        _cache["nc"] = _build()
    nc = _cache["nc"]

    in_maps = []
    for c in range(N_CORES):
        m = {
            "self_sh": self_p[c * NODES_PC : (c + 1) * NODES_PC],
            "neigh_sh": neigh_p[c * ROWS_PC : (c + 1) * ROWS_PC],
        }
        m.update(mk)
        in_maps.append(m)

    import os

    trace = os.environ.get("KERNEL_TRACE") == "1"
    res = run_bass_kernel_spmd(nc, in_maps, list(range(N_CORES)), trace=trace)
    _cache["last_result"] = res
    out = np.concatenate([res.results[c]["out"] for c in range(N_CORES)], axis=0)
    return out[:n].astype(np.float32)
